# revision 18
# baseline (speedup 1.0000x reference)
"""Trainium2 Bass kernel for BatchedExpertMoEDispatch.

Strategy (expert-parallel, sparse dispatch):
  - Host computes the routing table from (expert_ids, expert_weights):
    for each expert e the unique token list idx_e and combined coefficient
    coeff_e (duplicate (token, expert) slots merge by summing weights).
  - The token groups are "all-to-all"ed host-side (full-I/O contract): core e
    receives x.T gathered to its tokens [H, NCAP], its expert's gate/up/down
    weights in natural layout, and coeff_e.
  - Each core runs the full FFN for its expert on its tokens:
        gT = Wg.T @ xT ; uT = Wu.T @ xT          (PSUM, fp32 accum)
        hT = silu(gT) * uT                        (ACT + DVE)
        yT = Wd.T @ hT                            (PSUM, fp32 accum)
        outT = yT * coeff (broadcast over partitions)
    All activations live feature-major so every matmul operand is natural
    layout; matmuls run in float32r (full PE rate, fp32 I/O).
  - Host scatter-adds each core's outT back: out[idx_e] += outT[:, :n_e].T.

Capacity: NCAP tokens/core/round.  If any expert has more assigned tokens
(possible for adversarial routing distributions), the same compiled program
runs additional rounds on the remainder.
"""

import os
import sys

import numpy as np

for _p in ("/opt/trn_rl_repo", "/root/.axon_site/_ro/trn_rl_repo"):
    if os.path.isdir(_p) and _p not in sys.path:
        sys.path.append(_p)

import concourse.bacc as bacc
import concourse.mybir as mybir
import concourse.tile as tile
from concourse.bass_utils import run_bass_kernel_spmd

# Problem shapes (hardcoded per contract).
T, H, F, E, K = 4096, 1024, 2048, 8, 2
NCORES = 8
CKS = [512, 480]     # moving-operand chunks (fp32 max 512)
NCAP = sum(CKS)      # token capacity per core per round (>= seed-wise max)
NCHUNK = len(CKS)
COFF = [0, 512]      # chunk offsets
KH = H // 128        # 8  k-tiles over H
KF = F // 128        # 16 k-tiles over F
FP32 = mybir.dt.float32
FP32R = mybir.dt.float32r
MUL = mybir.AluOpType.mult

_PROGRAM = None

# Extra kwargs for run_bass_kernel_spmd — test harness pokes this to enable
# tracing; the grader path leaves it empty.
RUN_KWARGS: dict = {}
LAST_RESULTS = []


def build_program():
    """Build + compile the per-core SPMD FFN program (shared by all cores)."""
    nc = bacc.Bacc(
        "TRN2", target_bir_lowering=False, debug=False, num_devices=NCORES
    )
    xt_d = nc.dram_tensor("xt", [H, NCAP], FP32R, kind="ExternalInput")
    wg_d = nc.dram_tensor("wg", [H, F], FP32R, kind="ExternalInput")
    wu_d = nc.dram_tensor("wu", [H, F], FP32R, kind="ExternalInput")
    wd_d = nc.dram_tensor("wd", [F, H], FP32R, kind="ExternalInput")
    cf_d = nc.dram_tensor("cf", [1, NCAP], FP32, kind="ExternalInput")
    yt_d = nc.dram_tensor("yt", [H, NCAP], FP32, kind="ExternalOutput")

    with tile.TileContext(nc) as tc:
        from contextlib import ExitStack

        with ExitStack() as ctx:
            xt_pool = ctx.enter_context(tc.tile_pool(name="xt", bufs=1))
            ht_pool = ctx.enter_context(tc.tile_pool(name="ht", bufs=1))
            cf_pool = ctx.enter_context(tc.tile_pool(name="cf", bufs=1))
            wg_pool = ctx.enter_context(tc.tile_pool(name="wg", bufs=4))
            wu_pool = ctx.enter_context(tc.tile_pool(name="wu", bufs=4))
            wd_pool = ctx.enter_context(tc.tile_pool(name="wd", bufs=3))
            sl_pool = ctx.enter_context(tc.tile_pool(name="sl", bufs=4))
            ob_pool = ctx.enter_context(tc.tile_pool(name="ob", bufs=4))
            pg_pool = ctx.enter_context(tc.tile_pool(name="pg", bufs=3, space="PSUM"))
            pu_pool = ctx.enter_context(tc.tile_pool(name="pu", bufs=3, space="PSUM"))
            py_pool = ctx.enter_context(tc.tile_pool(name="py", bufs=2, space="PSUM"))

            # coeff, broadcast to all 128 partitions (needed only in phase 2;
            # issue on gpsimd's SWDGE queue to keep sync free for weights)
            cf_t = cf_pool.tile([128, NCAP], FP32, tag="cf")
            nc.gpsimd.dma_start(cf_t[:], cf_d.ap().partition_broadcast(128))

            # gate/up weight column loader: [128, KH*128], k-major free dim
            wgwu = {}

            def load_wgwu(f):
                wgt = wg_pool.tile([128, KH * 128], FP32R, tag="wg")
                wut = wu_pool.tile([128, KH * 128], FP32R, tag="wu")
                src_g = wg_d.ap()[:, f * 128 : (f + 1) * 128].rearrange(
                    "(k p) m -> p k m", p=128
                )
                src_u = wu_d.ap()[:, f * 128 : (f + 1) * 128].rearrange(
                    "(k p) m -> p k m", p=128
                )
                nc.sync.dma_start(
                    wgt[:].rearrange("p (k m) -> p k m", m=128), src_g
                )
                nc.sync.dma_start(
                    wut[:].rearrange("p (k m) -> p k m", m=128), src_u
                )
                wgwu[f] = (wgt, wut)

            # xT: per chunk, two k-half tiles [128, 4*ck] (k-major).
            # Sync-queue order = critical-first: xtc0 half 0, f0 weights,
            # xtc0 half 1, xtc1 halves, then the remaining weight columns.
            # One queue so nothing non-critical competes for HBM during
            # startup; the first matmul burst needs only xtc0h0 + wg0.
            xtc = {}

            def load_xt_half(ci, h):
                ck = CKS[ci]
                cs = COFF[ci]
                t = xt_pool.tile([128, 4 * ck], FP32R, tag=f"xtc{ci}_{h}")
                src = xt_d.ap()[
                    h * 512 : (h + 1) * 512, cs : cs + ck
                ].rearrange("(k p) t -> p k t", p=128)
                nc.sync.dma_start(
                    t[:].rearrange("p (k t) -> p k t", t=ck), src
                )
                xtc.setdefault(ci, []).append(t)

            load_xt_half(0, 0)
            load_wgwu(0)
            load_xt_half(0, 1)
            load_xt_half(1, 0)
            load_xt_half(1, 1)

            # Phase 1: hT[f] = silu(Wg[:,f].T @ xT) * (Wu[:,f].T @ xT)
            hts = []
            for f in range(KF):
                if f not in wgwu:
                    load_wgwu(f)
                wgt, wut = wgwu[f]
                ht = ht_pool.tile([128, NCAP], FP32R, tag=f"ht{f}")
                for ci in range(NCHUNK):
                    ck = CKS[ci]
                    cs, ce = COFF[ci], COFF[ci] + ck
                    pg = pg_pool.tile([128, ck], FP32, tag="pg")
                    pu = pu_pool.tile([128, ck], FP32, tag="pu")
                    # For the very first f-tile, interleave g/u in k-halves so
                    # the first burst only needs the first xT half-tile (the
                    # startup DMA stall shrinks by ~1MB of transfer time).
                    if f == 0 and ci == 0:
                        k_bursts = [(0, 4), (4, 8)]
                    else:
                        k_bursts = [(0, KH)]
                    for lo, hi in k_bursts:
                        for dst, w in ((pg, wgt), (pu, wut)):
                            for k in range(lo, hi):
                                nc.tensor.matmul(
                                    dst[:],
                                    w[:, k * 128 : (k + 1) * 128],
                                    xtc[ci][k // 4][
                                        :, (k % 4) * ck : (k % 4 + 1) * ck
                                    ],
                                    start=(k == 0),
                                    stop=(k == KH - 1),
                                )
                    sl = sl_pool.tile([128, ck], FP32, tag="sl")
                    nc.scalar.activation(
                        sl[:], pg[:], mybir.ActivationFunctionType.Sigmoid
                    )
                    nc.vector.tensor_tensor(sl[:], sl[:], pg[:], MUL)
                    nc.vector.tensor_tensor(ht[:, cs:ce], sl[:], pu[:], MUL)
                hts.append(ht)

            # Phase 2: yT[j] = Wd[:,j].T @ hT, scaled by coeff
            for j in range(KH):
                wdt = wd_pool.tile([128, KF * 128], FP32R, tag="wd")
                src_d = (
                    wd_d.ap()[:, j * 128 : (j + 1) * 128]
                    .rearrange("(k p) m -> p k m", p=128)
                )
                nc.sync.dma_start(
                    wdt[:].rearrange("p (k m) -> p k m", m=128), src_d
                )
                for ci in range(NCHUNK):
                    ck = CKS[ci]
                    cs, ce = COFF[ci], COFF[ci] + ck
                    py = py_pool.tile([128, ck], FP32, tag="py")
                    for kf in range(KF):
                        nc.tensor.matmul(
                            py[:],
                            wdt[:, kf * 128 : (kf + 1) * 128],
                            hts[kf][:, cs:ce],
                            start=(kf == 0),
                            stop=(kf == KF - 1),
                        )
                    # Final unit: split the coeff-mul + store into halves so
                    # the last DMA starts while the second half multiplies
                    # (shortens the kernel-end critical path slightly).
                    nsplit = 2 if (j == KH - 1 and ci == NCHUNK - 1) else 1
                    ob = ob_pool.tile([128, ck], FP32, tag="ob")
                    hw = ck // nsplit
                    for s in range(nsplit):
                        lo, hi = s * hw, (s + 1) * hw
                        nc.vector.tensor_tensor(
                            ob[:, lo:hi], py[:, lo:hi], cf_t[:, cs + lo : cs + hi], MUL
                        )
                        nc.scalar.dma_start(
                            yt_d.ap()[j * 128 : (j + 1) * 128, cs + lo : cs + hi],
                            ob[:, lo:hi],
                        )

    nc.compile()
    return nc


def _get_program():
    global _PROGRAM
    if _PROGRAM is None:
        _PROGRAM = build_program()
    return _PROGRAM


def kernel(x, expert_ids, expert_weights, gate_weights, up_weights, down_weights):
    x = np.ascontiguousarray(np.asarray(x, dtype=np.float32))
    expert_ids = np.asarray(expert_ids)
    expert_weights = np.asarray(expert_weights, dtype=np.float32)
    gate_weights = np.ascontiguousarray(np.asarray(gate_weights, dtype=np.float32))
    up_weights = np.ascontiguousarray(np.asarray(up_weights, dtype=np.float32))
    down_weights = np.ascontiguousarray(np.asarray(down_weights, dtype=np.float32))

    t_dim, h_dim = x.shape
    n_exp = gate_weights.shape[0]
    assert h_dim == H and gate_weights.shape[1:] == (H, F), (
        "program compiled for H=1024, F=2048"
    )
    assert n_exp == NCORES, "expert-parallel mapping assumes E == 8 cores"

    # Routing table: per-token combined coefficient per expert.
    coeff = np.zeros((t_dim, n_exp), np.float32)
    rows = np.arange(t_dim)
    for k in range(expert_ids.shape[1]):
        np.add.at(coeff, (rows, expert_ids[:, k]), expert_weights[:, k])

    idx_per_e = [np.nonzero(coeff[:, e])[0] for e in range(n_exp)]
    rounds = max(1, max((len(i) + NCAP - 1) // NCAP for i in idx_per_e))

    xT = np.ascontiguousarray(x.T)  # [H, T]
    nc = _get_program()

    out = np.zeros((t_dim, h_dim), np.float32)
    LAST_RESULTS.clear()
    for r in range(rounds):
        in_maps = []
        idx_r_per_e = []
        for e in range(n_exp):
            idx_r = idx_per_e[e][r * NCAP : (r + 1) * NCAP]
            idx_r_per_e.append(idx_r)
            xte = np.zeros((h_dim, NCAP), np.float32)
            cfe = np.zeros((1, NCAP), np.float32)
            if len(idx_r):
                xte[:, : len(idx_r)] = xT[:, idx_r]
                cfe[0, : len(idx_r)] = coeff[idx_r, e]
            in_maps.append(
                {
                    "xt": xte,
                    "wg": gate_weights[e],
                    "wu": up_weights[e],
                    "wd": down_weights[e],
                    "cf": cfe,
                }
            )
        res = run_bass_kernel_spmd(
            nc, in_maps, core_ids=list(range(NCORES)), **RUN_KWARGS
        )
        LAST_RESULTS.append(res)
        for e in range(n_exp):
            idx_r = idx_r_per_e[e]
            if len(idx_r):
                yt = res.results[e]["yt"]  # [H, NCAP], already coeff-scaled
                out[idx_r, :] += yt[:, : len(idx_r)].T
    return out


# revision 19
# speedup vs baseline: 1.1499x; 1.1499x over previous
"""Trainium2 Bass kernel for BatchedExpertMoEDispatch.

Strategy (expert-parallel, sparse dispatch):
  - Host computes the routing table from (expert_ids, expert_weights):
    for each expert e the unique token list idx_e and combined coefficient
    coeff_e (duplicate (token, expert) slots merge by summing weights).
  - The token groups are "all-to-all"ed host-side (full-I/O contract): core e
    receives x.T gathered to its tokens [H, NCAP], its expert's gate/up/down
    weights in natural layout, and coeff_e.
  - Each core runs the full FFN for its expert on its tokens:
        gT = Wg.T @ xT ; uT = Wu.T @ xT          (PSUM, fp32 accum)
        hT = silu(gT) * uT                        (ACT + DVE)
        yT = Wd.T @ hT                            (PSUM, fp32 accum)
        outT = yT * coeff (broadcast over partitions)
    All activations live feature-major so every matmul operand is natural
    layout; matmuls run in float32r (full PE rate, fp32 I/O).
  - Host scatter-adds each core's outT back: out[idx_e] += outT[:, :n_e].T.

Capacity: NCAP tokens/core/round.  If any expert has more assigned tokens
(possible for adversarial routing distributions), the same compiled program
runs additional rounds on the remainder.
"""

import os
import sys

import numpy as np

for _p in ("/opt/trn_rl_repo", "/root/.axon_site/_ro/trn_rl_repo"):
    if os.path.isdir(_p) and _p not in sys.path:
        sys.path.append(_p)

import concourse.bacc as bacc
import concourse.mybir as mybir
import concourse.tile as tile
from concourse.bass_utils import run_bass_kernel_spmd

# Problem shapes (hardcoded per contract).
T, H, F, E, K = 4096, 1024, 2048, 8, 2
NCORES = 8
CKS = [512, 472]     # moving-operand chunks (fp32 max 512)
NCAP = sum(CKS)      # token capacity per core per round (>= seed-wise max)
NCHUNK = len(CKS)
COFF = [0, 512]      # chunk offsets
KH = H // 128        # 8  k-tiles over H
KF = F // 128        # 16 k-tiles over F
FP32 = mybir.dt.float32
FP32R = mybir.dt.float32r
MUL = mybir.AluOpType.mult

_PROGRAM = None

# Extra kwargs for run_bass_kernel_spmd — test harness pokes this to enable
# tracing; the grader path leaves it empty.
RUN_KWARGS: dict = {}
LAST_RESULTS = []


def build_program():
    """Build + compile the per-core SPMD FFN program (shared by all cores)."""
    nc = bacc.Bacc(
        "TRN2", target_bir_lowering=False, debug=False, num_devices=NCORES
    )
    xt_d = nc.dram_tensor("xt", [H, NCAP], FP32R, kind="ExternalInput")
    wg_d = nc.dram_tensor("wg", [H, F], FP32R, kind="ExternalInput")
    wu_d = nc.dram_tensor("wu", [H, F], FP32R, kind="ExternalInput")
    wd_d = nc.dram_tensor("wd", [F, H], FP32R, kind="ExternalInput")
    cf_d = nc.dram_tensor("cf", [1, NCAP], FP32, kind="ExternalInput")
    yt_d = nc.dram_tensor("yt", [H, NCAP], FP32, kind="ExternalOutput")

    with tile.TileContext(nc) as tc:
        from contextlib import ExitStack

        with ExitStack() as ctx:
            xt_pool = ctx.enter_context(tc.tile_pool(name="xt", bufs=1))
            ht_pool = ctx.enter_context(tc.tile_pool(name="ht", bufs=1))
            cf_pool = ctx.enter_context(tc.tile_pool(name="cf", bufs=1))
            wg_pool = ctx.enter_context(tc.tile_pool(name="wg", bufs=4))
            wu_pool = ctx.enter_context(tc.tile_pool(name="wu", bufs=4))
            wd_pool = ctx.enter_context(tc.tile_pool(name="wd", bufs=3))
            sl_pool = ctx.enter_context(tc.tile_pool(name="sl", bufs=4))
            ob_pool = ctx.enter_context(tc.tile_pool(name="ob", bufs=4))
            pg_pool = ctx.enter_context(tc.tile_pool(name="pg", bufs=3, space="PSUM"))
            pu_pool = ctx.enter_context(tc.tile_pool(name="pu", bufs=3, space="PSUM"))
            py_pool = ctx.enter_context(tc.tile_pool(name="py", bufs=2, space="PSUM"))

            # coeff, broadcast to all 128 partitions (needed only in phase 2;
            # issue on gpsimd's SWDGE queue to keep sync free for weights)
            cf_t = cf_pool.tile([128, NCAP], FP32, tag="cf")
            nc.gpsimd.dma_start(cf_t[:], cf_d.ap().partition_broadcast(128))

            # gate/up weight column loader: [128, KH*128], k-major free dim
            wgwu = {}

            def load_wgwu(f):
                wgt = wg_pool.tile([128, KH * 128], FP32R, tag="wg")
                wut = wu_pool.tile([128, KH * 128], FP32R, tag="wu")
                src_g = wg_d.ap()[:, f * 128 : (f + 1) * 128].rearrange(
                    "(k p) m -> p k m", p=128
                )
                src_u = wu_d.ap()[:, f * 128 : (f + 1) * 128].rearrange(
                    "(k p) m -> p k m", p=128
                )
                nc.sync.dma_start(
                    wgt[:].rearrange("p (k m) -> p k m", m=128), src_g
                )
                nc.sync.dma_start(
                    wut[:].rearrange("p (k m) -> p k m", m=128), src_u
                )
                wgwu[f] = (wgt, wut)

            # xT: per chunk, two k-half tiles [128, 4*ck] (k-major).
            # Sync-queue order = critical-first: xtc0 half 0, f0 weights,
            # xtc0 half 1, xtc1 halves, then the remaining weight columns.
            # One queue so nothing non-critical competes for HBM during
            # startup; the first matmul burst needs only xtc0h0 + wg0.
            xtc = {}

            def load_xt_half(ci, h):
                ck = CKS[ci]
                cs = COFF[ci]
                t = xt_pool.tile([128, 4 * ck], FP32R, tag=f"xtc{ci}_{h}")
                src = xt_d.ap()[
                    h * 512 : (h + 1) * 512, cs : cs + ck
                ].rearrange("(k p) t -> p k t", p=128)
                nc.sync.dma_start(
                    t[:].rearrange("p (k t) -> p k t", t=ck), src
                )
                xtc.setdefault(ci, []).append(t)

            # f0's first burst g(k0..3) needs only xtc0h0 + wg0 (1.5MB);
            # g(k4..7) adds xtc0h1; the u groups then need wu0.
            load_xt_half(0, 0)
            wgt0 = wg_pool.tile([128, KH * 128], FP32R, tag="wg", name="wgt0")
    
            nc.sync.dma_start(
                wgt0[:].rearrange("p (k m) -> p k m", m=128),
                wg_d.ap()[:, 0:128].rearrange("(k p) m -> p k m", p=128),
            )
            load_xt_half(0, 1)
            wut0 = wu_pool.tile([128, KH * 128], FP32R, tag="wu", name="wut0")
            nc.sync.dma_start(
                wut0[:].rearrange("p (k m) -> p k m", m=128),
                wu_d.ap()[:, 0:128].rearrange("(k p) m -> p k m", p=128),
            )
            wgwu[0] = (wgt0, wut0)
            load_xt_half(1, 0)
            load_xt_half(1, 1)

            # Phase 1: hT[f] = silu(Wg[:,f].T @ xT) * (Wu[:,f].T @ xT)
            hts = []
            for f in range(KF):
                if f not in wgwu:
                    load_wgwu(f)
                wgt, wut = wgwu[f]
                ht = ht_pool.tile([128, NCAP], FP32R, tag=f"ht{f}")
                for ci in range(NCHUNK):
                    ck = CKS[ci]
                    cs, ce = COFF[ci], COFF[ci] + ck
                    pg = pg_pool.tile([128, ck], FP32, tag="pg")
                    pu = pu_pool.tile([128, ck], FP32, tag="pu")
                    # For the very first f-tile, interleave g/u in k-halves so
                    # the first burst only needs the first xT half-tile (the
                    # startup DMA stall shrinks by ~1MB of transfer time).
                    if f == 0 and ci == 0:
                        bursts = [(pg, wgt, 0, 4), (pg, wgt, 4, 8),
                                  (pu, wut, 0, 8)]
                    else:
                        bursts = [(pg, wgt, 0, KH), (pu, wut, 0, KH)]
                    for dst, w, lo, hi in bursts:
                        if True:
                            for k in range(lo, hi):
                                nc.tensor.matmul(
                                    dst[:],
                                    w[:, k * 128 : (k + 1) * 128],
                                    xtc[ci][k // 4][
                                        :, (k % 4) * ck : (k % 4 + 1) * ck
                                    ],
                                    start=(k == 0),
                                    stop=(k == KH - 1),
                                )
                    sl = sl_pool.tile([128, ck], FP32, tag="sl")
                    nc.scalar.activation(
                        sl[:], pg[:], mybir.ActivationFunctionType.Sigmoid
                    )
                    nc.vector.tensor_tensor(sl[:], sl[:], pg[:], MUL)
                    nc.vector.tensor_tensor(ht[:, cs:ce], sl[:], pu[:], MUL)
                hts.append(ht)

            # Phase 2: yT[j] = Wd[:,j].T @ hT, scaled by coeff
            for j in range(KH):
                wdt = wd_pool.tile([128, KF * 128], FP32R, tag="wd")
                src_d = (
                    wd_d.ap()[:, j * 128 : (j + 1) * 128]
                    .rearrange("(k p) m -> p k m", p=128)
                )
                nc.sync.dma_start(
                    wdt[:].rearrange("p (k m) -> p k m", m=128), src_d
                )
                for ci in range(NCHUNK):
                    ck = CKS[ci]
                    cs, ce = COFF[ci], COFF[ci] + ck
                    py = py_pool.tile([128, ck], FP32, tag="py")
                    for kf in range(KF):
                        nc.tensor.matmul(
                            py[:],
                            wdt[:, kf * 128 : (kf + 1) * 128],
                            hts[kf][:, cs:ce],
                            start=(kf == 0),
                            stop=(kf == KF - 1),
                        )
                    # Final unit: split the coeff-mul + store into halves so
                    # the last DMA starts while the second half multiplies
                    # (shortens the kernel-end critical path slightly).
                    nsplit = 2 if (j == KH - 1 and ci == NCHUNK - 1) else 1
                    ob = ob_pool.tile([128, ck], FP32, tag="ob")
                    hw = ck // nsplit
                    for s in range(nsplit):
                        lo, hi = s * hw, (s + 1) * hw
                        nc.vector.tensor_tensor(
                            ob[:, lo:hi], py[:, lo:hi], cf_t[:, cs + lo : cs + hi], MUL
                        )
                        nc.scalar.dma_start(
                            yt_d.ap()[j * 128 : (j + 1) * 128, cs + lo : cs + hi],
                            ob[:, lo:hi],
                        )

    nc.compile()
    return nc


def _get_program():
    global _PROGRAM
    if _PROGRAM is None:
        _PROGRAM = build_program()
    return _PROGRAM


def kernel(x, expert_ids, expert_weights, gate_weights, up_weights, down_weights):
    x = np.ascontiguousarray(np.asarray(x, dtype=np.float32))
    expert_ids = np.asarray(expert_ids)
    expert_weights = np.asarray(expert_weights, dtype=np.float32)
    gate_weights = np.ascontiguousarray(np.asarray(gate_weights, dtype=np.float32))
    up_weights = np.ascontiguousarray(np.asarray(up_weights, dtype=np.float32))
    down_weights = np.ascontiguousarray(np.asarray(down_weights, dtype=np.float32))

    t_dim, h_dim = x.shape
    n_exp = gate_weights.shape[0]
    assert h_dim == H and gate_weights.shape[1:] == (H, F), (
        "program compiled for H=1024, F=2048"
    )
    assert n_exp == NCORES, "expert-parallel mapping assumes E == 8 cores"

    # Routing table: per-token combined coefficient per expert.
    coeff = np.zeros((t_dim, n_exp), np.float32)
    rows = np.arange(t_dim)
    for k in range(expert_ids.shape[1]):
        np.add.at(coeff, (rows, expert_ids[:, k]), expert_weights[:, k])

    idx_per_e = [np.nonzero(coeff[:, e])[0] for e in range(n_exp)]
    rounds = max(1, max((len(i) + NCAP - 1) // NCAP for i in idx_per_e))

    xT = np.ascontiguousarray(x.T)  # [H, T]
    nc = _get_program()

    out = np.zeros((t_dim, h_dim), np.float32)
    LAST_RESULTS.clear()
    for r in range(rounds):
        in_maps = []
        idx_r_per_e = []
        for e in range(n_exp):
            idx_r = idx_per_e[e][r * NCAP : (r + 1) * NCAP]
            idx_r_per_e.append(idx_r)
            xte = np.zeros((h_dim, NCAP), np.float32)
            cfe = np.zeros((1, NCAP), np.float32)
            if len(idx_r):
                xte[:, : len(idx_r)] = xT[:, idx_r]
                cfe[0, : len(idx_r)] = coeff[idx_r, e]
            in_maps.append(
                {
                    "xt": xte,
                    "wg": gate_weights[e],
                    "wu": up_weights[e],
                    "wd": down_weights[e],
                    "cf": cfe,
                }
            )
        res = run_bass_kernel_spmd(
            nc, in_maps, core_ids=list(range(NCORES)), **RUN_KWARGS
        )
        LAST_RESULTS.append(res)
        for e in range(n_exp):
            idx_r = idx_r_per_e[e]
            if len(idx_r):
                yt = res.results[e]["yt"]  # [H, NCAP], already coeff-scaled
                out[idx_r, :] += yt[:, : len(idx_r)].T
    return out


# revision 20
# speedup vs baseline: 1.1832x; 1.0289x over previous
"""Trainium2 Bass kernel for BatchedExpertMoEDispatch.

Strategy (expert-parallel, sparse dispatch):
  - Host computes the routing table from (expert_ids, expert_weights):
    for each expert e the unique token list idx_e and combined coefficient
    coeff_e (duplicate (token, expert) slots merge by summing weights).
  - The token groups are "all-to-all"ed host-side (full-I/O contract): core e
    receives x.T gathered to its tokens [H, NCAP], its expert's gate/up/down
    weights in natural layout, and coeff_e.
  - Each core runs the full FFN for its expert on its tokens:
        gT = Wg.T @ xT ; uT = Wu.T @ xT          (PSUM, fp32 accum)
        hT = silu(gT) * uT                        (ACT + DVE)
        yT = Wd.T @ hT                            (PSUM, fp32 accum)
        outT = yT * coeff (broadcast over partitions)
    All activations live feature-major so every matmul operand is natural
    layout; matmuls run in float32r (full PE rate, fp32 I/O).
  - Host scatter-adds each core's outT back: out[idx_e] += outT[:, :n_e].T.

Capacity: NCAP tokens/core/round.  If any expert has more assigned tokens
(possible for adversarial routing distributions), the same compiled program
runs additional rounds on the remainder.
"""

import os
import sys

import numpy as np

for _p in ("/opt/trn_rl_repo", "/root/.axon_site/_ro/trn_rl_repo"):
    if os.path.isdir(_p) and _p not in sys.path:
        sys.path.append(_p)

import concourse.bacc as bacc
import concourse.mybir as mybir
import concourse.tile as tile
from concourse.bass_utils import run_bass_kernel_spmd

# Problem shapes (hardcoded per contract).
T, H, F, E, K = 4096, 1024, 2048, 8, 2
NCORES = 8
CKS = [512, 480]     # moving-operand chunks (fp32 max 512)
NCAP = sum(CKS)      # token capacity per core per round (>= seed-wise max)
NCHUNK = len(CKS)
COFF = [0, 512]      # chunk offsets
KH = H // 128        # 8  k-tiles over H
KF = F // 128        # 16 k-tiles over F
FP32 = mybir.dt.float32
FP32R = mybir.dt.float32r
MUL = mybir.AluOpType.mult

_PROGRAM = None

# Extra kwargs for run_bass_kernel_spmd — test harness pokes this to enable
# tracing; the grader path leaves it empty.
RUN_KWARGS: dict = {}
LAST_RESULTS = []


def build_program():
    """Build + compile the per-core SPMD FFN program (shared by all cores)."""
    nc = bacc.Bacc(
        "TRN2", target_bir_lowering=False, debug=False, num_devices=NCORES
    )
    xt_d = nc.dram_tensor("xt", [H, NCAP], FP32R, kind="ExternalInput")
    wg_d = nc.dram_tensor("wg", [H, F], FP32R, kind="ExternalInput")
    wu_d = nc.dram_tensor("wu", [H, F], FP32R, kind="ExternalInput")
    wd_d = nc.dram_tensor("wd", [F, H], FP32R, kind="ExternalInput")
    cf_d = nc.dram_tensor("cf", [1, NCAP], FP32, kind="ExternalInput")
    yt_d = nc.dram_tensor("yt", [H, NCAP], FP32, kind="ExternalOutput")

    with tile.TileContext(nc) as tc:
        from contextlib import ExitStack

        with ExitStack() as ctx:
            xt_pool = ctx.enter_context(tc.tile_pool(name="xt", bufs=1))
            ht_pool = ctx.enter_context(tc.tile_pool(name="ht", bufs=1))
            cf_pool = ctx.enter_context(tc.tile_pool(name="cf", bufs=1))
            wg_pool = ctx.enter_context(tc.tile_pool(name="wg", bufs=4))
            wu_pool = ctx.enter_context(tc.tile_pool(name="wu", bufs=4))
            wd_pool = ctx.enter_context(tc.tile_pool(name="wd", bufs=3))
            sl_pool = ctx.enter_context(tc.tile_pool(name="sl", bufs=4))
            ob_pool = ctx.enter_context(tc.tile_pool(name="ob", bufs=4))
            pg_pool = ctx.enter_context(tc.tile_pool(name="pg", bufs=3, space="PSUM"))
            pu_pool = ctx.enter_context(tc.tile_pool(name="pu", bufs=3, space="PSUM"))
            py_pool = ctx.enter_context(tc.tile_pool(name="py", bufs=2, space="PSUM"))

            # coeff, broadcast to all 128 partitions (needed only in phase 2;
            # issue on gpsimd's SWDGE queue to keep sync free for weights)
            cf_t = cf_pool.tile([128, NCAP], FP32, tag="cf")
            nc.gpsimd.dma_start(cf_t[:], cf_d.ap().partition_broadcast(128))

            # gate/up weight column loader: [128, KH*128], k-major free dim
            wgwu = {}

            def load_wgwu(f):
                wgt = wg_pool.tile([128, KH * 128], FP32R, tag="wg")
                wut = wu_pool.tile([128, KH * 128], FP32R, tag="wu")
                src_g = wg_d.ap()[:, f * 128 : (f + 1) * 128].rearrange(
                    "(k p) m -> p k m", p=128
                )
                src_u = wu_d.ap()[:, f * 128 : (f + 1) * 128].rearrange(
                    "(k p) m -> p k m", p=128
                )
                nc.sync.dma_start(
                    wgt[:].rearrange("p (k m) -> p k m", m=128), src_g
                )
                nc.sync.dma_start(
                    wut[:].rearrange("p (k m) -> p k m", m=128), src_u
                )
                wgwu[f] = (wgt, wut)

            # xT: per chunk, two k-half tiles [128, 4*ck] (k-major).
            # Sync-queue order = critical-first: xtc0 half 0, f0 weights,
            # xtc0 half 1, xtc1 halves, then the remaining weight columns.
            # One queue so nothing non-critical competes for HBM during
            # startup; the first matmul burst needs only xtc0h0 + wg0.
            xtc = {}

            def load_xt_half(ci, h):
                ck = CKS[ci]
                cs = COFF[ci]
                t = xt_pool.tile([128, 4 * ck], FP32R, tag=f"xtc{ci}_{h}")
                src = xt_d.ap()[
                    h * 512 : (h + 1) * 512, cs : cs + ck
                ].rearrange("(k p) t -> p k t", p=128)
                nc.sync.dma_start(
                    t[:].rearrange("p (k t) -> p k t", t=ck), src
                )
                xtc.setdefault(ci, []).append(t)

            load_xt_half(0, 0)
            load_wgwu(0)
            load_xt_half(0, 1)
            load_xt_half(1, 0)
            load_xt_half(1, 1)

            # Phase 1: hT[f] = silu(Wg[:,f].T @ xT) * (Wu[:,f].T @ xT)
            hts = []
            for f in range(KF):
                if f not in wgwu:
                    load_wgwu(f)
                wgt, wut = wgwu[f]
                ht = ht_pool.tile([128, NCAP], FP32R, tag=f"ht{f}")
                for ci in range(NCHUNK):
                    ck = CKS[ci]
                    cs, ce = COFF[ci], COFF[ci] + ck
                    pg = pg_pool.tile([128, ck], FP32, tag="pg")
                    pu = pu_pool.tile([128, ck], FP32, tag="pu")
                    # For the very first f-tile, interleave g/u in k-halves so
                    # the first burst only needs the first xT half-tile (the
                    # startup DMA stall shrinks by ~1MB of transfer time).
                    if f == 0 and ci == 0:
                        k_bursts = [(0, 4), (4, 8)]
                    else:
                        k_bursts = [(0, KH)]
                    for lo, hi in k_bursts:
                        for dst, w in ((pg, wgt), (pu, wut)):
                            for k in range(lo, hi):
                                nc.tensor.matmul(
                                    dst[:],
                                    w[:, k * 128 : (k + 1) * 128],
                                    xtc[ci][k // 4][
                                        :, (k % 4) * ck : (k % 4 + 1) * ck
                                    ],
                                    start=(k == 0),
                                    stop=(k == KH - 1),
                                )
                    sl = sl_pool.tile([128, ck], FP32, tag="sl")
                    nc.scalar.activation(
                        sl[:], pg[:], mybir.ActivationFunctionType.Sigmoid
                    )
                    nc.vector.tensor_tensor(sl[:], sl[:], pg[:], MUL)
                    nc.vector.tensor_tensor(ht[:, cs:ce], sl[:], pu[:], MUL)
                hts.append(ht)

            # Phase 2: yT[j] = Wd[:,j].T @ hT, scaled by coeff
            for j in range(KH):
                wdt = wd_pool.tile([128, KF * 128], FP32R, tag="wd")
                src_d = (
                    wd_d.ap()[:, j * 128 : (j + 1) * 128]
                    .rearrange("(k p) m -> p k m", p=128)
                )
                nc.sync.dma_start(
                    wdt[:].rearrange("p (k m) -> p k m", m=128), src_d
                )
                for ci in range(NCHUNK):
                    ck = CKS[ci]
                    cs, ce = COFF[ci], COFF[ci] + ck
                    py = py_pool.tile([128, ck], FP32, tag="py")
                    for kf in range(KF):
                        nc.tensor.matmul(
                            py[:],
                            wdt[:, kf * 128 : (kf + 1) * 128],
                            hts[kf][:, cs:ce],
                            start=(kf == 0),
                            stop=(kf == KF - 1),
                        )
                    # Final unit: split the coeff-mul + store into halves so
                    # the last DMA starts while the second half multiplies
                    # (shortens the kernel-end critical path slightly).
                    nsplit = 2 if (j == KH - 1 and ci == NCHUNK - 1) else 1
                    ob = ob_pool.tile([128, ck], FP32, tag="ob")
                    hw = ck // nsplit
                    for s in range(nsplit):
                        lo, hi = s * hw, (s + 1) * hw
                        nc.vector.tensor_tensor(
                            ob[:, lo:hi], py[:, lo:hi], cf_t[:, cs + lo : cs + hi], MUL
                        )
                        nc.scalar.dma_start(
                            yt_d.ap()[j * 128 : (j + 1) * 128, cs + lo : cs + hi],
                            ob[:, lo:hi],
                        )

    nc.compile()
    return nc


def _get_program():
    global _PROGRAM
    if _PROGRAM is None:
        _PROGRAM = build_program()
    return _PROGRAM


def kernel(x, expert_ids, expert_weights, gate_weights, up_weights, down_weights):
    x = np.ascontiguousarray(np.asarray(x, dtype=np.float32))
    expert_ids = np.asarray(expert_ids)
    expert_weights = np.asarray(expert_weights, dtype=np.float32)
    gate_weights = np.ascontiguousarray(np.asarray(gate_weights, dtype=np.float32))
    up_weights = np.ascontiguousarray(np.asarray(up_weights, dtype=np.float32))
    down_weights = np.ascontiguousarray(np.asarray(down_weights, dtype=np.float32))

    t_dim, h_dim = x.shape
    n_exp = gate_weights.shape[0]
    assert h_dim == H and gate_weights.shape[1:] == (H, F), (
        "program compiled for H=1024, F=2048"
    )
    assert n_exp == NCORES, "expert-parallel mapping assumes E == 8 cores"

    # Routing table: per-token combined coefficient per expert.
    coeff = np.zeros((t_dim, n_exp), np.float32)
    rows = np.arange(t_dim)
    for k in range(expert_ids.shape[1]):
        np.add.at(coeff, (rows, expert_ids[:, k]), expert_weights[:, k])

    idx_per_e = [np.nonzero(coeff[:, e])[0] for e in range(n_exp)]
    rounds = max(1, max((len(i) + NCAP - 1) // NCAP for i in idx_per_e))

    xT = np.ascontiguousarray(x.T)  # [H, T]
    nc = _get_program()

    out = np.zeros((t_dim, h_dim), np.float32)
    LAST_RESULTS.clear()
    for r in range(rounds):
        in_maps = []
        idx_r_per_e = []
        for e in range(n_exp):
            idx_r = idx_per_e[e][r * NCAP : (r + 1) * NCAP]
            idx_r_per_e.append(idx_r)
            xte = np.zeros((h_dim, NCAP), np.float32)
            cfe = np.zeros((1, NCAP), np.float32)
            if len(idx_r):
                xte[:, : len(idx_r)] = xT[:, idx_r]
                cfe[0, : len(idx_r)] = coeff[idx_r, e]
            in_maps.append(
                {
                    "xt": xte,
                    "wg": gate_weights[e],
                    "wu": up_weights[e],
                    "wd": down_weights[e],
                    "cf": cfe,
                }
            )
        res = run_bass_kernel_spmd(
            nc, in_maps, core_ids=list(range(NCORES)), **RUN_KWARGS
        )
        LAST_RESULTS.append(res)
        for e in range(n_exp):
            idx_r = idx_r_per_e[e]
            if len(idx_r):
                yt = res.results[e]["yt"]  # [H, NCAP], already coeff-scaled
                out[idx_r, :] += yt[:, : len(idx_r)].T
    return out
